# revision 1
# baseline (speedup 1.0000x reference)
"""Trainium2 Bass kernel for nn_BlockSelfAttentionModule (v5, bf16, software-pipelined).

Same math/layout as v4 (see its docstring). v5 replaces the barriered
For_i body with tc.For_i_pipelined stages so consecutive iterations
overlap with point-to-point waits:

  stage 0 load : q DMA (sync)                          -> qsb
  stage 1 mm   : 4 bf16 matmuls (PE) + 4 PSUM->SBUF casts (ACT) -> gsb
  stage 2 skew : 8 ring + 4 SWDGE gathers, 4 DVE plain skews,
                 8 DVE voice selects                   -> ts12, vt12
  stage 3 out  : 12 broadcast adds (DVE 9 / POOL 3, b0 first)
                 + 1 merged store (sync, all 4 batches)

Steady-state per-iteration cost ~ max engine budget:
  HWDGE 11 dma_starts, DVE ~6.7us, POOL ~5.8us, ACT ~2us, PE ~1.4us.
"""

import os
import sys

for _p in ("/opt/trn_rl_repo", "/root/.axon_site/_ro/trn_rl_repo"):
    if os.path.isdir(_p) and _p not in sys.path:
        sys.path.insert(0, _p)

import contextlib

import numpy as np
import ml_dtypes

import concourse.bass as bass
import concourse.bacc as bacc
import concourse.mybir as mybir
import concourse.tile as tile
from concourse.bass_utils import run_bass_kernel_spmd

E, H, DI, DO, F = 16, 8, 8, 8, 48
L = F * DI  # 384
B = 4
NJ = 3 * B  # 12
GP = 1536
TP = NJ * F  # 576
NCORES = 8
BF = mybir.dt.bfloat16
F32 = mybir.dt.float32
U8 = mybir.dt.uint8

_prog_cache = {}

MSK = np.ascontiguousarray(
    (np.arange(128)[:, None] % 8 == np.arange(8)[None, :]).astype(np.uint8)
)

SKEW_PLAIN = (0, 4, 8, 12)  # DVE
SKEW_DMA_SYNC = (1, 3, 5, 9)
SKEW_DMA_SCALAR = (7, 11, 13, 15)
SKEW_DMA_GPSIMD = (2, 6, 10, 14)
ADD_POOL = {(1, 0), (2, 0), (3, 0)}  # (b, c) on POOL; rest DVE


def build_program(loop_n=None, unroll=4, cfg=None):
    cfg = cfg or {}
    dma_sync = cfg.get("dma_sync", SKEW_DMA_SYNC)
    dma_scalar = cfg.get("dma_scalar", SKEW_DMA_SCALAR)
    dma_gpsimd = cfg.get("dma_gpsimd", SKEW_DMA_GPSIMD)
    add_pool = cfg.get("add_pool", ADD_POOL)
    plain_eng = cfg.get("plain_eng", "vector")
    n_stores = cfg.get("n_stores", 1)
    trip = loop_n if loop_n is not None else 1

    nc = bacc.Bacc("TRN2", target_bir_lowering=False, debug=False)
    winp = nc.dram_tensor("winp", [48, 384], BF, kind="ExternalInput")
    qinp = nc.dram_tensor("qinp", [48, 512], BF, kind="ExternalInput")
    mk = nc.dram_tensor("mk", [128, 8], U8, kind="ExternalInput")
    out = nc.dram_tensor("out", [B, L, L], BF, kind="ExternalOutput")

    with tile.TileContext(nc) as tc, contextlib.ExitStack() as ctx:
        const_pool = ctx.enter_context(tc.tile_pool(name="const", bufs=1))
        zp_pool = ctx.enter_context(tc.tile_pool(name="zp", bufs=8, space="PSUM"))
        osb_pool = ctx.enter_context(tc.tile_pool(name="osb", bufs=2))

        wsb = const_pool.tile([48, 384], BF)
        msk = const_pool.tile([128, 8], U8)
        nc.scalar.dma_start(wsb[:], winp[:])
        nc.gpsimd.dma_start(msk[:], mk[:])

        def stage_load(pipe, iv):
            qsb = pipe.intermediate_tile([48, 512], BF, name="qsb")
            nc.sync.dma_start(qsb[:], qinp[:])
            return qsb

        def stage_mm(pipe, iv, qsb):
            gsb = pipe.intermediate_tile([128, GP], BF, name="gsb")
            z_tiles = []
            for b in range(B):
                lhsT = bass.AP(qsb.tensor, 128 * b, [[512, 48], [1, 128]])
                z = zp_pool.tile([128, 384], F32, tag="z")
                nc.tensor.matmul(z[:], lhsT, wsb[:])
                z_tiles.append(z)
            for b in range(B):
                dst = bass.AP(gsb.tensor, 384 * b, [[GP, 128], [1, 384]])
                nc.scalar.copy(dst, z_tiles[b][:])
            return gsb

        def stage_skew(pipe, iv, gsb):
            ts12 = pipe.intermediate_tile([128, TP], BF, name="ts12")
            vt12 = pipe.intermediate_tile([128, NJ * 8], BF, name="vt12")
            for fp in range(16):
                if fp in SKEW_PLAIN:
                    continue
                src = bass.AP(
                    gsb.tensor, 8 * fp * GP + fp, [[GP, 8], [128, NJ], [1, F]]
                )
                dst = bass.AP(ts12.tensor, 8 * fp * TP, [[TP, 8], [F, NJ], [1, F]])
                if fp in dma_sync:
                    nc.sync.dma_start(dst, src)
                elif fp in dma_scalar:
                    nc.scalar.dma_start(dst, src)
                else:
                    assert fp in dma_gpsimd
                    nc.gpsimd.dma_start(dst, src)
            for di in range(8):
                data = bass.AP(
                    gsb.tensor, 64 + 8 * di,
                    [[GP, 128], [384, B], [1, 8], [128, 3]],
                )
                mask = bass.AP(msk.tensor, di, [[8, 128], [0, B], [0, 8], [0, 3]])
                vout = bass.AP(
                    vt12.tensor, 0, [[NJ * 8, 128], [3, B], [NJ, 8], [1, 3]]
                )
                nc.vector.copy_predicated(vout, mask, data)
            for fp in SKEW_PLAIN:
                src = bass.AP(
                    gsb.tensor, 8 * fp * GP + fp, [[GP, 8], [128, NJ], [1, F]]
                )
                dst = bass.AP(ts12.tensor, 8 * fp * TP, [[TP, 8], [F, NJ], [1, F]])
                if plain_eng == "vector":
                    nc.vector.tensor_copy(dst, src)
                else:
                    nc.scalar.copy(dst, src)
            return (ts12, vt12)

        add_merge = cfg.get("add_merge", False)

        def stage_out(pipe, iv, tv):
            ts12, vt12 = tv
            osb = osb_pool.tile([128, B * 3 * L], BF, tag="osb")
            for b in range(B):
                if add_merge and b > 0:
                    # one 4-dim add covering all 3 chunks of this batch (DVE)
                    t_b = bass.AP(
                        ts12.tensor, 3 * b * F + 47,
                        [[TP, 128], [F, 3], [-1, F], [0, 8]],
                    )
                    v_b = bass.AP(
                        vt12.tensor, 3 * b,
                        [[NJ * 8, 128], [1, 3], [0, F], [NJ, 8]],
                    )
                    o_ap = bass.AP(
                        osb.tensor, 3 * b * L,
                        [[B * 3 * L, 128], [L, 3], [8, F], [1, 8]],
                    )
                    nc.vector.tensor_add(o_ap, t_b, v_b)
                else:
                    for c in range(3):
                        j = 3 * b + c
                        t_b = bass.AP(
                            ts12.tensor, j * F + 47, [[TP, 128], [-1, F], [0, 8]]
                        )
                        v_b = bass.AP(
                            vt12.tensor, j, [[NJ * 8, 128], [0, F], [NJ, 8]]
                        )
                        o_ap = bass.AP(
                            osb.tensor, j * L, [[B * 3 * L, 128], [8, F], [1, 8]]
                        )
                        eng = (
                            nc.gpsimd
                            if (add_merge or (b, c) in add_pool)
                            else nc.vector
                        )
                        eng.tensor_add(o_ap, t_b, v_b)
                if n_stores == 2 and b % 2 == 1:
                    half = b // 2
                    st_src = bass.AP(
                        osb.tensor, half * 6 * L, [[B * 3 * L, 128], [L, 6], [1, L]]
                    )
                    st_dst = bass.AP(
                        out, half * 2 * L * L, [[L, 128], [128 * L, 6], [1, L]]
                    )
                    (nc.sync if half == 0 else nc.scalar).dma_start(st_dst, st_src)
                elif n_stores == 1 and b == 3:
                    st_src = bass.AP(
                        osb.tensor, 0, [[B * 3 * L, 128], [L, 12], [1, L]]
                    )
                    st_dst = bass.AP(out, 0, [[L, 128], [128 * L, 12], [1, L]])
                    nc.sync.dma_start(st_dst, st_src)

        tc.For_i_pipelined(
            [stage_load, stage_mm, stage_skew, stage_out],
            0,
            trip,
            unroll=unroll,
        )

    nc.compile()
    return nc


def _get_program():
    if "nc" not in _prog_cache:
        _prog_cache["nc"] = build_program()
    return _prog_cache["nc"]


def make_core_inputs(q, r_voice, e_past, e_future):
    q = np.asarray(q, dtype=np.float32)
    qb = q.reshape(B, H, L, E)
    in_maps = []
    for h in range(NCORES):
        w = np.zeros((48, 384), np.float32)
        gfull = np.empty((E, 95), np.float32)
        gfull[:, 0:47] = e_future[1:48, :, h][::-1].T
        gfull[:, 47:95] = e_past[:, :, h].T
        u = r_voice[:, :, :, h].reshape(DI * DO, E).T  # (E, 64)
        for c in range(3):
            blk = np.zeros((E, 128), np.float32)
            blk[:, 0:63] = gfull[:, 16 * c:16 * c + 63]
            blk[:, 64:128] = u
            w[c * 16:(c + 1) * 16, 128 * c:128 * (c + 1)] = blk
        qh = qb[:, h]  # (B, L, E)
        qt = qh.reshape(B, 3, 128, E).transpose(1, 3, 0, 2).reshape(48, 512)
        in_maps.append(
            {
                "winp": w.astype(ml_dtypes.bfloat16),
                "qinp": np.ascontiguousarray(qt).astype(ml_dtypes.bfloat16),
                "mk": MSK,
            }
        )
    return in_maps


def kernel(q, flipped_masks, r_voice, e_past, e_future):
    q = np.asarray(q, dtype=np.float32)
    r_voice = np.asarray(r_voice, dtype=np.float32)
    e_past = np.asarray(e_past, dtype=np.float32)
    e_future = np.asarray(e_future, dtype=np.float32)

    nc = _get_program()
    in_maps = make_core_inputs(q, r_voice, e_past, e_future)
    res = run_bass_kernel_spmd(nc, in_maps, core_ids=list(range(NCORES)))

    out = np.empty((B * H, L, L), dtype=np.float32)
    for h in range(NCORES):
        out_h = np.asarray(res.results[h]["out"]).astype(np.float32)
        for b in range(B):
            out[b * H + h] = out_h[b]
    return out

